# revision 11
# baseline (speedup 1.0000x reference)
"""Trainium2 Bass kernel for nn_BiAttention (sparse_attention).

Math: att[b,l,m] = idot[b,l] + s_m[b,m] (rank-1 + mask bias), so row
softmax over m is l-invariant: output_one[b,l,:] = v_b, and
output_two = softmax_l(idot) @ inp2. Output row blocks [N, 4*Ld, d]:
    [0:2048]    inp2 = input @ W_in2.T + b_in2        (device, full rank)
    [2048:4096] v_b broadcast                          (host)
    [4096:6144] inp2 * v_b                             (device, full rank)
    [6144:8192] (output_two * v_b) broadcast           (host)
All rank-1 reductions (~0.1% of FLOPs) run on host in f64; the device
does only the dense work, transposed (features on partitions):
    ps[oc,g]  = sum_k W2T[k,oc]^T @ inT[k,g]       (PE, bf16, f32 PSUM)
    o1T[oc,g] = ps + b                             (ACT evict)
    o3T[oc,g] = ps*v[oc] + (b*v)[oc]               (DVE evict)

Scheduling notes (measured):
  - DMA issue costs ~600ns per instruction on the issuing ring,
    regardless of size -> batch writes per (g, oc-half): 16 write DMAs.
  - g0 runs k-major so a (w2t-k, inT-g0-k) arrival bundle (384KB,
    ~1.1us) feeds 8 matmuls (~1.7us): PE starts ~3us after the first
    DMA instead of waiting for full tensors.
  - g1-3 run oc-major so the 8 PSUM banks close staggered through the
    block and ACT/DVE evictions never gate the next block's bank reuse.
  - smalls (bias/v columns) go on the gpsimd SWDGE ring to keep the SP
    ring's early slots for the w2t/inT bundles.
"""

import numpy as np
import ml_dtypes

import concourse.bass as bass
import concourse.tile as tile
from concourse import bacc, bass_isa, mybir
from concourse.bass_utils import run_bass_kernel_spmd

F32 = mybir.dt.float32
BF16 = mybir.dt.bfloat16
OP = mybir.AluOpType
IDENT = mybir.ActivationFunctionType.Identity

P = 128
BSZ, LD, LM, HID = 8, 2048, 512, 1024
KT = HID // P          # 8 hidden-dim chunks
GT = 4                 # l groups of 512
GL = LD // GT          # 512
N_CORES = 8

_NC_CACHE = None


def _build_nc():
    nc = bacc.Bacc("TRN2", target_bir_lowering=False, num_devices=N_CORES)

    inT_d = nc.dram_tensor("inT", [HID, LD], BF16, kind="ExternalInput").ap()
    w2t_d = nc.dram_tensor("w2t", [HID, HID], BF16, kind="ExternalInput").ap()
    bi2c_d = nc.dram_tensor("bi2c", [P, KT], F32, kind="ExternalInput").ap()
    vcol_d = nc.dram_tensor("vcol", [P, KT], F32, kind="ExternalInput").ap()
    bvcol_d = nc.dram_tensor("bvcol", [P, KT], F32, kind="ExternalInput").ap()
    o1T_d = nc.dram_tensor("o1T", [HID, LD], BF16, kind="ExternalOutput").ap()
    o3T_d = nc.dram_tensor("o3T", [HID, LD], BF16, kind="ExternalOutput").ap()

    with tile.TileContext(nc) as tc:
        with (
            tc.tile_pool(name="const", bufs=1) as cpool,
            tc.tile_pool(name="w", bufs=1) as wpool,
            tc.tile_pool(name="inp", bufs=1) as inpool,
            tc.tile_pool(name="o1s", bufs=2) as o1pool,
            tc.tile_pool(name="o3s", bufs=2) as o3pool,
            tc.tile_pool(name="psmm", bufs=8, space="PSUM") as psmm,
        ):
            bi2c = cpool.tile([P, KT], F32, tag="bi2c")
            nc.gpsimd.dma_start(bi2c[:], bi2c_d[:])
            vcol = cpool.tile([P, KT], F32, tag="vcol")
            nc.gpsimd.dma_start(vcol[:], vcol_d[:])
            bvcol = cpool.tile([P, KT], F32, tag="bvcol")
            nc.gpsimd.dma_start(bvcol[:], bvcol_d[:])
            warmsb = cpool.tile([P, GL], BF16, tag="warm")
            nc.vector.memset(warmsb[:], 0.0)

            w2sb = wpool.tile([P, KT, HID], BF16, tag="w2sb")
            insb = inpool.tile([P, KT, LD], BF16, tag="insb")

            # weight reads ride the ACT ring (idle until evictions) so the
            # SP ring's first slot goes to inT-g0-k0; both rings' first
            # bundles arrive concurrently.
            for k in range(KT):
                nc.scalar.dma_start(w2sb[:, k, :], w2t_d[k * P:(k + 1) * P, :])
                nc.sync.dma_start(insb[:, k, 0:GL],
                                  inT_d[k * P:(k + 1) * P, 0:GL])
            for g in range(1, GT):
                nc.sync.dma_start(
                    insb[:, :, g * GL:(g + 1) * GL],
                    inT_d[:, g * GL:(g + 1) * GL].rearrange(
                        "(k p) x -> p k x", p=P),
                )

            # PE p-state warmup: ~12 tiny self-contained matmuls on a
            # memset tile keep the PE busy while the first bundles land,
            # so the real stream starts at the full 2.4GHz clock instead
            # of ramping through pstate-mid for its first ~3us.
            warm_ps = psmm.tile([P, GL], F32, tag="mm", name="warm")
            for w in range(10):
                nc.tensor.matmul(warm_ps[:], warmsb[:, 0:P], warmsb[:],
                                 start=True, stop=True)

            def emit_evict(g, oc, ps, o1g, o3g):
                nc.scalar.activation(o1g[:, oc, :], ps[:], IDENT,
                                     bias=bi2c[:, oc:oc + 1])
                nc.vector.tensor_scalar(o3g[:, oc, :], ps[:],
                                        vcol[:, oc:oc + 1],
                                        bvcol[:, oc:oc + 1],
                                        OP.mult, OP.add)

            def emit_write(g, oc0, oc1, o1g, o3g):
                for dst, src in ((o1T_d, o1g), (o3T_d, o3g)):
                    nc.sync.dma_start(
                        dst[oc0 * P:oc1 * P,
                            g * GL:(g + 1) * GL].rearrange(
                                "(k p) x -> p k x", p=P),
                        src[:, oc0:oc1, :],
                    )

            for g in range(GT):
                pst = [psmm.tile([P, GL], F32, tag="mm", name=f"mm{g}_{oc}")
                       for oc in range(KT)]
                o1g = o1pool.tile([P, KT, GL], BF16, tag="o1s", name=f"o1_{g}")
                o3g = o3pool.tile([P, KT, GL], BF16, tag="o3s", name=f"o3_{g}")
                # last block: oc-pair write granularity to shrink the tail
                wsplits = {3: 4, 5: 6, 7: 8} if g == GT - 1 else {3: 4, 7: 8}
                wprev = 0
                if g == 0:
                    # k-major: bundle-k feeds 8 matmuls; banks close on k7
                    for k in range(KT):
                        for oc in range(KT):
                            nc.tensor.matmul(
                                pst[oc][:],
                                w2sb[:, k, oc * P:(oc + 1) * P],
                                insb[:, k, g * GL:(g + 1) * GL],
                                start=(k == 0), stop=(k == KT - 1),
                            )
                            if k == KT - 1:
                                emit_evict(g, oc, pst[oc], o1g, o3g)
                                if oc in wsplits:
                                    emit_write(g, wprev, wsplits[oc],
                                               o1g, o3g)
                                    wprev = wsplits[oc]
                else:
                    # oc-major: banks close staggered through the block
                    for oc in range(KT):
                        for k in range(KT):
                            nc.tensor.matmul(
                                pst[oc][:],
                                w2sb[:, k, oc * P:(oc + 1) * P],
                                insb[:, k, g * GL:(g + 1) * GL],
                                start=(k == 0), stop=(k == KT - 1),
                            )
                        emit_evict(g, oc, pst[oc], o1g, o3g)
                        if oc in wsplits:
                            emit_write(g, wprev, wsplits[oc], o1g, o3g)
                            wprev = wsplits[oc]

    nc.finalize()
    return nc


def _get_nc():
    global _NC_CACHE
    if _NC_CACHE is None:
        _NC_CACHE = _build_nc()
    return _NC_CACHE


def _softmax(x):
    x = x - x.max(axis=-1, keepdims=True)
    e = np.exp(x)
    return e / e.sum(axis=-1, keepdims=True)


def kernel(**inputs) -> np.ndarray:
    nc = _get_nc()
    bf16 = ml_dtypes.bfloat16

    inp = np.asarray(inputs["input"], np.float32)
    mem = np.asarray(inputs["memory"], np.float32)
    mask = np.asarray(inputs["mask"], np.float32)
    w_in1 = np.asarray(inputs["w_in1"], np.float32).reshape(HID)
    w_mem1 = np.asarray(inputs["w_mem1"], np.float32).reshape(HID)
    W_in2 = np.asarray(inputs["W_in2"], np.float32)
    b_in2 = np.asarray(inputs["b_in2"], np.float32).reshape(HID)
    W_mem2 = np.asarray(inputs["W_mem2"], np.float32)
    b_mem2 = np.asarray(inputs["b_mem2"], np.float32).reshape(HID)

    # ---- host: rank-1 side chains in f64 ----
    inp64 = inp.astype(np.float64)
    mem64 = mem.astype(np.float64)
    idot = inp64 @ w_in1.astype(np.float64)            # [N, Ld]
    e = _softmax(idot)
    q = np.einsum('bl,bld->bd', e, inp64)              # [N, d]
    ot2 = q @ W_in2.astype(np.float64).T + b_in2       # [N, d]
    s_m = mem64 @ w_mem1.astype(np.float64)            # [N, Lm]
    att = s_m - 1e30 * (1.0 - mask.astype(np.float64))
    w1 = _softmax(att)
    p = np.einsum('bm,bmd->bd', w1, mem64)             # [N, d]
    v = p @ W_mem2.astype(np.float64).T + b_mem2       # [N, d]
    u = (ot2 * v).astype(np.float32)                   # [N, d]
    v32 = v.astype(np.float32)

    w2t = W_in2.T.astype(bf16)
    bi2c = np.ascontiguousarray(b_in2.reshape(KT, P).T)

    in_maps = []
    for b in range(N_CORES):
        vb = v32[b]
        in_maps.append({
            "inT": inp[b].T.astype(bf16),
            "w2t": w2t,
            "bi2c": bi2c,
            "vcol": np.ascontiguousarray(vb.reshape(KT, P).T),
            "bvcol": np.ascontiguousarray((b_in2 * vb).reshape(KT, P).T),
        })

    res = run_bass_kernel_spmd(nc, in_maps, core_ids=list(range(N_CORES)))

    out = np.empty((BSZ, 4 * LD, HID), np.float32)
    for b in range(N_CORES):
        r = res.results[b]
        out[b, 0:LD] = r["o1T"].T
        out[b, LD:2 * LD] = v32[b]
        out[b, 2 * LD:3 * LD] = r["o3T"].T
        out[b, 3 * LD:4 * LD] = u[b]
    return out


# revision 13
# speedup vs baseline: 1.0023x; 1.0023x over previous
"""Trainium2 Bass kernel for nn_BiAttention (sparse_attention).

Math: att[b,l,m] = idot[b,l] + s_m[b,m] (rank-1 + mask bias), so row
softmax over m is l-invariant: output_one[b,l,:] = v_b, and
output_two = softmax_l(idot) @ inp2. Output row blocks [N, 4*Ld, d]:
    [0:2048]    inp2 = input @ W_in2.T + b_in2        (device, full rank)
    [2048:4096] v_b broadcast                          (host)
    [4096:6144] inp2 * v_b                             (device, full rank)
    [6144:8192] (output_two * v_b) broadcast           (host)
All rank-1 reductions (~0.1% of FLOPs) run on host in f64; the device
does only the dense work, transposed (features on partitions):
    ps[oc,g]  = sum_k W2T[k,oc]^T @ inT[k,g]       (PE, bf16, f32 PSUM)
    o1T[oc,g] = ps + b                             (ACT evict)
    o3T[oc,g] = ps*v[oc] + (b*v)[oc]               (DVE evict)

Scheduling notes (measured):
  - DMA issue costs ~600ns per instruction on the issuing ring,
    regardless of size -> batch writes per (g, oc-half): 16 write DMAs.
  - g0 runs k-major so a (w2t-k, inT-g0-k) arrival bundle (384KB,
    ~1.1us) feeds 8 matmuls (~1.7us): PE starts ~3us after the first
    DMA instead of waiting for full tensors.
  - g1-3 run oc-major so the 8 PSUM banks close staggered through the
    block and ACT/DVE evictions never gate the next block's bank reuse.
  - smalls (bias/v columns) go on the gpsimd SWDGE ring to keep the SP
    ring's early slots for the w2t/inT bundles.
"""

import numpy as np
import ml_dtypes

import concourse.bass as bass
import concourse.tile as tile
from concourse import bacc, bass_isa, mybir
from concourse.bass_utils import run_bass_kernel_spmd

F32 = mybir.dt.float32
BF16 = mybir.dt.bfloat16
OP = mybir.AluOpType
IDENT = mybir.ActivationFunctionType.Identity

P = 128
BSZ, LD, LM, HID = 8, 2048, 512, 1024
KT = HID // P          # 8 hidden-dim chunks
GT = 4                 # l groups of 512
GL = LD // GT          # 512
N_CORES = 8

_NC_CACHE = None


def _build_nc():
    nc = bacc.Bacc("TRN2", target_bir_lowering=False, num_devices=N_CORES)

    inT_d = nc.dram_tensor("inT", [HID, LD], BF16, kind="ExternalInput").ap()
    w2t_d = nc.dram_tensor("w2t", [HID, HID], BF16, kind="ExternalInput").ap()
    bi2c_d = nc.dram_tensor("bi2c", [P, KT], F32, kind="ExternalInput").ap()
    vcol_d = nc.dram_tensor("vcol", [P, KT], F32, kind="ExternalInput").ap()
    bvcol_d = nc.dram_tensor("bvcol", [P, KT], F32, kind="ExternalInput").ap()
    o1T_d = nc.dram_tensor("o1T", [HID, LD], BF16, kind="ExternalOutput").ap()
    o3T_d = nc.dram_tensor("o3T", [HID, LD], BF16, kind="ExternalOutput").ap()

    with tile.TileContext(nc) as tc:
        with (
            tc.tile_pool(name="const", bufs=1) as cpool,
            tc.tile_pool(name="w", bufs=1) as wpool,
            tc.tile_pool(name="inp", bufs=1) as inpool,
            tc.tile_pool(name="o1s", bufs=2) as o1pool,
            tc.tile_pool(name="o3s", bufs=2) as o3pool,
            tc.tile_pool(name="psmm", bufs=8, space="PSUM") as psmm,
        ):
            bi2c = cpool.tile([P, KT], F32, tag="bi2c")
            nc.gpsimd.dma_start(bi2c[:], bi2c_d[:])
            vcol = cpool.tile([P, KT], F32, tag="vcol")
            nc.gpsimd.dma_start(vcol[:], vcol_d[:])
            bvcol = cpool.tile([P, KT], F32, tag="bvcol")
            nc.gpsimd.dma_start(bvcol[:], bvcol_d[:])
            warmsb = cpool.tile([P, GL], BF16, tag="warm")
            nc.vector.memset(warmsb[:], 0.0)

            w2sb = wpool.tile([P, KT, HID], BF16, tag="w2sb")
            insb = inpool.tile([P, KT, LD], BF16, tag="insb")

            # weight reads ride the ACT ring (idle until evictions) so the
            # SP ring's first slot goes to inT-g0-k0; both rings' first
            # bundles arrive concurrently.
            for k in range(KT):
                nc.scalar.dma_start(w2sb[:, k, :], w2t_d[k * P:(k + 1) * P, :])
                nc.sync.dma_start(insb[:, k, 0:GL],
                                  inT_d[k * P:(k + 1) * P, 0:GL])
            for g in range(1, GT):
                nc.sync.dma_start(
                    insb[:, :, g * GL:(g + 1) * GL],
                    inT_d[:, g * GL:(g + 1) * GL].rearrange(
                        "(k p) x -> p k x", p=P),
                )

            # PE p-state warmup: ~12 tiny self-contained matmuls on a
            # memset tile keep the PE busy while the first bundles land,
            # so the real stream starts at the full 2.4GHz clock instead
            # of ramping through pstate-mid for its first ~3us.
            warm_ps = psmm.tile([P, GL], F32, tag="mm", name="warm")
            for w in range(5):
                nc.tensor.matmul(warm_ps[:], warmsb[:, 0:P], warmsb[:],
                                 start=True, stop=True)

            def emit_evict(g, oc, ps, o1g, o3g):
                nc.scalar.activation(o1g[:, oc, :], ps[:], IDENT,
                                     bias=bi2c[:, oc:oc + 1])
                nc.vector.tensor_scalar(o3g[:, oc, :], ps[:],
                                        vcol[:, oc:oc + 1],
                                        bvcol[:, oc:oc + 1],
                                        OP.mult, OP.add)

            def emit_write(g, oc0, oc1, o1g, o3g):
                for dst, src in ((o1T_d, o1g), (o3T_d, o3g)):
                    nc.sync.dma_start(
                        dst[oc0 * P:oc1 * P,
                            g * GL:(g + 1) * GL].rearrange(
                                "(k p) x -> p k x", p=P),
                        src[:, oc0:oc1, :],
                    )

            for g in range(GT):
                pst = [psmm.tile([P, GL], F32, tag="mm", name=f"mm{g}_{oc}")
                       for oc in range(KT)]
                o1g = o1pool.tile([P, KT, GL], BF16, tag="o1s", name=f"o1_{g}")
                o3g = o3pool.tile([P, KT, GL], BF16, tag="o3s", name=f"o3_{g}")
                # last block: oc-pair write granularity to shrink the tail
                wsplits = ({3: 4, 5: 6, 6: 7, 7: 8} if g == GT - 1
                           else {3: 4, 7: 8})
                wprev = 0
                if g == 0:
                    # k-major: bundle-k feeds 8 matmuls; banks close on k7
                    for k in range(KT):
                        for oc in range(KT):
                            nc.tensor.matmul(
                                pst[oc][:],
                                w2sb[:, k, oc * P:(oc + 1) * P],
                                insb[:, k, g * GL:(g + 1) * GL],
                                start=(k == 0), stop=(k == KT - 1),
                            )
                            if k == KT - 1:
                                emit_evict(g, oc, pst[oc], o1g, o3g)
                                if oc in wsplits:
                                    emit_write(g, wprev, wsplits[oc],
                                               o1g, o3g)
                                    wprev = wsplits[oc]
                else:
                    # oc-major: banks close staggered through the block
                    for oc in range(KT):
                        for k in range(KT):
                            nc.tensor.matmul(
                                pst[oc][:],
                                w2sb[:, k, oc * P:(oc + 1) * P],
                                insb[:, k, g * GL:(g + 1) * GL],
                                start=(k == 0), stop=(k == KT - 1),
                            )
                        emit_evict(g, oc, pst[oc], o1g, o3g)
                        if oc in wsplits:
                            emit_write(g, wprev, wsplits[oc], o1g, o3g)
                            wprev = wsplits[oc]

    nc.finalize()
    return nc


def _get_nc():
    global _NC_CACHE
    if _NC_CACHE is None:
        _NC_CACHE = _build_nc()
    return _NC_CACHE


def _softmax(x):
    x = x - x.max(axis=-1, keepdims=True)
    e = np.exp(x)
    return e / e.sum(axis=-1, keepdims=True)


def kernel(**inputs) -> np.ndarray:
    nc = _get_nc()
    bf16 = ml_dtypes.bfloat16

    inp = np.asarray(inputs["input"], np.float32)
    mem = np.asarray(inputs["memory"], np.float32)
    mask = np.asarray(inputs["mask"], np.float32)
    w_in1 = np.asarray(inputs["w_in1"], np.float32).reshape(HID)
    w_mem1 = np.asarray(inputs["w_mem1"], np.float32).reshape(HID)
    W_in2 = np.asarray(inputs["W_in2"], np.float32)
    b_in2 = np.asarray(inputs["b_in2"], np.float32).reshape(HID)
    W_mem2 = np.asarray(inputs["W_mem2"], np.float32)
    b_mem2 = np.asarray(inputs["b_mem2"], np.float32).reshape(HID)

    # ---- host: rank-1 side chains in f64 ----
    inp64 = inp.astype(np.float64)
    mem64 = mem.astype(np.float64)
    idot = inp64 @ w_in1.astype(np.float64)            # [N, Ld]
    e = _softmax(idot)
    q = np.einsum('bl,bld->bd', e, inp64)              # [N, d]
    ot2 = q @ W_in2.astype(np.float64).T + b_in2       # [N, d]
    s_m = mem64 @ w_mem1.astype(np.float64)            # [N, Lm]
    att = s_m - 1e30 * (1.0 - mask.astype(np.float64))
    w1 = _softmax(att)
    p = np.einsum('bm,bmd->bd', w1, mem64)             # [N, d]
    v = p @ W_mem2.astype(np.float64).T + b_mem2       # [N, d]
    u = (ot2 * v).astype(np.float32)                   # [N, d]
    v32 = v.astype(np.float32)

    w2t = W_in2.T.astype(bf16)
    bi2c = np.ascontiguousarray(b_in2.reshape(KT, P).T)

    in_maps = []
    for b in range(N_CORES):
        vb = v32[b]
        in_maps.append({
            "inT": inp[b].T.astype(bf16),
            "w2t": w2t,
            "bi2c": bi2c,
            "vcol": np.ascontiguousarray(vb.reshape(KT, P).T),
            "bvcol": np.ascontiguousarray((b_in2 * vb).reshape(KT, P).T),
        })

    res = run_bass_kernel_spmd(nc, in_maps, core_ids=list(range(N_CORES)))

    out = np.empty((BSZ, 4 * LD, HID), np.float32)
    for b in range(N_CORES):
        r = res.results[b]
        out[b, 0:LD] = r["o1T"].T
        out[b, LD:2 * LD] = v32[b]
        out[b, 2 * LD:3 * LD] = r["o3T"].T
        out[b, 3 * LD:4 * LD] = u[b]
    return out
